# revision 1
# baseline (speedup 1.0000x reference)
"""Multi-head attention on 8 Trainium2 NeuronCores.

Sharding: data-parallel over batch (4) x tensor-parallel over head-groups (2).
Core c handles batch c//2, heads [8*(c%2), 8*(c%2)+8). Each core computes its
partial out-projection (over its 512 channels); host sums the pair per batch.

Device-side layout (per core, all matmuls float32r = fp32 with 11-bit mantissa):
  Q^T, K^T  [512, 2048]  (channel-major)  <- proj with W chunks stationary
  V         [2048, 520]  (token-major, 65 cols/head: 64 V + ones column)
  scores^T  [s', t] per (head, t-chunk) -> exp on ACT -> P^T (f32r)
  P^T *= mask^T (bf16 resident block)  on DVE
  attn@V: lhsT=V_h (stationary) rhs=P^T -> out^T [65, t] with denominator row
  normalize via DVE recip + K=1 broadcast matmul
  out-proj: lhsT=Wo^T chunks, rhs=attn_norm^T -> out_partial^T [1024, 2048]
"""
import sys

sys.path.insert(0, "/opt/trn_rl_repo")

import numpy as np
import ml_dtypes

import concourse.bass as bass
import concourse.mybir as mybir
import concourse.tile as tile
from concourse import bacc
from concourse.bass_utils import run_bass_kernel_spmd

D_MODEL = 1024
NUM_HEADS = 16
DK = 64
B, S = 4, 2048
NCORES = 8
OG = 512            # channels per head-group
HPG = 8             # heads per group
IC = D_MODEL // 128  # 8 contraction chunks
F32 = mybir.dt.float32
F32R = mybir.dt.float32r
BF16 = mybir.dt.bfloat16
PT_BF16 = False         # P^T and V in bf16 (faster DVE mask-mul); False = f32r


def round_f32r(x: np.ndarray) -> np.ndarray:
    """Round fp32 to fp32r (11-bit mantissa, round-to-nearest-even)."""
    b = np.ascontiguousarray(x, dtype=np.float32).view(np.uint32).astype(np.uint64)
    lsb = (b >> 12) & 1
    b = (b + 0x7FF + lsb) & 0xFFFFF000
    return (b & 0xFFFFFFFF).astype(np.uint32).view(np.float32)


def build_module():
    nc = bacc.Bacc("TRN2", target_bir_lowering=False, debug=False,
                   num_devices=NCORES)
    XQT = nc.dram_tensor("XQT", [D_MODEL, S], F32R, kind="ExternalInput").ap()
    XKT = nc.dram_tensor("XKT", [D_MODEL, S], F32R, kind="ExternalInput").ap()
    XVT = nc.dram_tensor("XVT", [D_MODEL, S], F32R, kind="ExternalInput").ap()
    WQT = nc.dram_tensor("WQT", [D_MODEL, OG], F32R, kind="ExternalInput").ap()
    WKT = nc.dram_tensor("WKT", [D_MODEL, OG], F32R, kind="ExternalInput").ap()
    WVT = nc.dram_tensor("WVT", [D_MODEL, OG], F32R, kind="ExternalInput").ap()
    WOT = nc.dram_tensor("WOT", [OG, D_MODEL], F32R, kind="ExternalInput").ap()
    MASKT = nc.dram_tensor("MASKT", [S, S], BF16, kind="ExternalInput").ap()
    BQ = nc.dram_tensor("BQ", [OG], F32, kind="ExternalInput").ap()
    BK = nc.dram_tensor("BK", [OG], F32, kind="ExternalInput").ap()
    BV = nc.dram_tensor("BV", [1, OG], F32R, kind="ExternalInput").ap()
    ONES = nc.dram_tensor("ONES", [128, 128], F32R, kind="ExternalInput").ap()
    OUTT = nc.dram_tensor("OUTT", [D_MODEL, S], F32, kind="ExternalOutput").ap()

    Exp = mybir.ActivationFunctionType.Exp
    Ident = mybir.ActivationFunctionType.Identity

    with tile.TileContext(nc) as tc:
        with tc.tile_pool(name="persist", bufs=1) as pp, \
             tc.tile_pool(name="qkpool", bufs=1) as qkp:
            # projection outputs, resident through the whole kernel
            qT = [qkp.tile([128, S], F32R, name=f"qT{j}") for j in range(4)]
            kT = [qkp.tile([128, S], F32R, name=f"kT{j}") for j in range(4)]
            PT_DT = BF16 if PT_BF16 else F32R
            bq_t = pp.tile([128, 4], F32, name="bq_t")
            bk_t = pp.tile([128, 4], F32, name="bk_t")
            bv_t = pp.tile([1, OG], F32R, name="bv_t")
            ones_t = pp.tile([128, 128], F32R, name="ones_t")

            nc.sync.dma_start(out=bq_t[:], in_=BQ.rearrange("(j p) -> p j", p=128))
            nc.sync.dma_start(out=bk_t[:], in_=BK.rearrange("(j p) -> p j", p=128))
            nc.sync.dma_start(out=bv_t[:], in_=BV)
            nc.sync.dma_start(out=ones_t[:], in_=ONES)

            # ---------------- Phase A: projections ----------------
            vtp = tc.alloc_tile_pool(name="vtpool", bufs=1)
            vt = [vtp.tile([128, HPG * 65], PT_DT, name=f"vt{j}")
                  for j in range(16)]
            with tc.tile_pool(name="wpool", bufs=1) as wp, \
                 tc.tile_pool(name="xpool", bufs=9) as xp, \
                 tc.tile_pool(name="psA", bufs=1, space="PSUM") as psA:
                wq = [wp.tile([128, OG], F32R, name=f"wq{i}") for i in range(IC)]
                wk = [wp.tile([128, OG], F32R, name=f"wk{i}") for i in range(IC)]
                wv = [wp.tile([128, OG], F32R, tag=f"wq{i}", name=f"wv{i}")
                      for i in range(IC)]

                def wsl(wts, i, lo, hi):
                    return wts[i][:, lo:hi]

                # Q^T and K^T: out^T[o, t], W chunks stationary; x resident
                engs = (nc.sync, nc.scalar, nc.gpsimd)
                for xdram, wdram, wts, outts, bias in (
                        (XKT, WKT, wk, kT, bk_t), (XQT, WQT, wq, qT, bq_t)):
                    xts = []
                    for i in range(IC):
                        xt = xp.tile([128, S], F32R, tag="xt", name=f"xt{i}")
                        engs[i % 3].dma_start(
                            out=xt[:], in_=xdram[i * 128:(i + 1) * 128, :])
                        engs[(i + 1) % 3].dma_start(
                            out=wts[i][:], in_=wdram[i * 128:(i + 1) * 128, :])
                        xts.append(xt)
                    for rnd in range(2):
                        tcs = (2 * rnd, 2 * rnd + 1)
                        psums = {}
                        for och in range(4):
                            for ti, t in enumerate(tcs):
                                psums[(och, t)] = psA.tile(
                                    [128, 512], F32, tag=f"pj{och}_{ti}",
                                    name=f"pj{och}_{ti}")
                        for i in range(IC):
                            for och in range(4):
                                for t in tcs:
                                    nc.tensor.matmul(
                                        psums[(och, t)],
                                        wsl(wts, i, och * 128, (och + 1) * 128),
                                        xts[i][:, t * 512:(t + 1) * 512],
                                        start=(i == 0), stop=(i == IC - 1))
                        for och in range(4):
                            for t in tcs:
                                nc.scalar.activation(
                                    outts[och][:, t * 512:(t + 1) * 512],
                                    psums[(och, t)], Ident,
                                    bias=bias[:, och:och + 1])

                # -------- V projection (same pools, no release barrier) ----
                for tch in range(16):
                    ocol = vt[tch][:].rearrange("p (h e) -> p h e", h=HPG)[:, :, 64:65]
                    nc.vector.memset(ocol if PT_BF16 else ocol.bitcast(F32), 1.0)
                xts = []
                for i in range(IC):
                    xt = xp.tile([128, S], F32R, tag="xt", name=f"xv{i}")
                    engs[i % 3].dma_start(
                        out=xt[:], in_=XVT[i * 128:(i + 1) * 128, :])
                    engs[(i + 1) % 3].dma_start(
                        out=wv[i][:], in_=WVT[i * 128:(i + 1) * 128, :])
                    xts.append(xt)
                for vrnd in range(2):
                    tchs = list(range(8 * vrnd, 8 * vrnd + 8))
                    psums = {tch: psA.tile(
                        [128, 512], F32, tag=f"pj{(tch % 8) // 2}_{tch % 2}",
                        name=f"pv{tch % 8}") for tch in tchs}
                    for i in range(IC):
                        for tch in tchs:
                            nc.tensor.matmul(
                                psums[tch], xts[i][:, tch * 128:(tch + 1) * 128],
                                wsl(wv, i, 0, OG), start=(i == 0), stop=False)
                    for tch in tchs:
                        # bias row: ones[t] (x) bv[o], closes the accum group
                        nc.tensor.matmul(
                            psums[tch], ones_t[0:1, 0:128], bv_t[0:1, :],
                            start=False, stop=True)
                        nc.vector.tensor_copy(
                            vt[tch][:].rearrange(
                                "p (h e) -> p h e", h=HPG)[:, :, 0:64],
                            psums[tch][:].rearrange("p (h d) -> p h d", h=HPG))

            # ---------------- Phase B/C: attention + out-proj ----------------
            with tc.tile_pool(name="wopool", bufs=1) as wop, \
                 tc.tile_pool(name="mpool", bufs=3) as mp, \
                 tc.tile_pool(name="ptpool", bufs=7) as ptp, \
                 tc.tile_pool(name="workpool", bufs=1) as wkp, \
                 tc.tile_pool(name="onrmpool", bufs=2) as onp, \
                 tc.tile_pool(name="stgpool", bufs=2) as sgp, \
                 tc.tile_pool(name="psB", bufs=1, space="PSUM") as psB:
                wo = [wop.tile([128, D_MODEL], F32R, name=f"wo{j}")
                      for j in range(4)]
                for j in range(4):
                    nc.sync.dma_start(out=wo[j][:],
                                      in_=WOT[j * 128:(j + 1) * 128, :])
                for t in range(4):
                    mhs = []
                    for half in range(2):
                        mh = mp.tile([128, 8 * 512], BF16, tag="mask",
                                     name="mask_t")
                        mhs.append(mh)
                        for s8 in range(8):
                            sc = half * 8 + s8
                            nc.sync.dma_start(
                                out=mh[:, s8 * 512:(s8 + 1) * 512],
                                in_=MASKT[sc * 128:(sc + 1) * 128,
                                          t * 512:(t + 1) * 512])
                    onrm = [onp.tile([128, 512], F32R, tag=f"onrm{j}",
                                     name=f"onrm{j}") for j in range(4)]
                    for h in range(HPG):
                        ht, ho = h // 2, (h % 2) * 64
                        ps_o = psB.tile([65, 512], F32, tag=f"o{h % 2}",
                                        name=f"o{h % 2}")
                        # software pipeline over 8 batches of 2 s'-chunks:
                        # scores(j) -> [attnV(j-1)] -> exp(j) -> mask-mul(j)
                        # pipeline: scores(j) -> attnV(j-1) -> exp(j) -> mask(j)
                        pts = []
                        for j in range(8):
                            ps_s = psB.tile([128, 1024], F32, tag=f"s{j % 2}",
                                            name=f"s{j % 2}")
                            pt = ptp.tile([128, 1024], PT_DT, tag="pt",
                                          name="pt")
                            pts.append(pt)
                            for u in range(2):
                                sc = 2 * j + u
                                nc.tensor.matmul(
                                    ps_s[:, u * 512:(u + 1) * 512],
                                    kT[ht][ho:ho + 64, sc * 128:(sc + 1) * 128],
                                    qT[ht][ho:ho + 64, t * 512:(t + 1) * 512],
                                    start=True, stop=True)
                            if j > 0:
                                for u in range(2):
                                    sc = 2 * (j - 1) + u
                                    nc.tensor.matmul(
                                        ps_o, vt[sc][:, h * 65:h * 65 + 65],
                                        pts[j - 1][:, u * 512:(u + 1) * 512],
                                        start=(sc == 0), stop=False,
                                        skip_group_check=True)
                            nc.scalar.activation(pt[:], ps_s, Exp, scale=0.125)
                            nc.vector.tensor_mul(
                                pt[:], pt[:],
                                mhs[j // 4][:, (j % 4) * 1024:(j % 4 + 1) * 1024])
                        for u in range(2):
                            sc = 14 + u
                            nc.tensor.matmul(
                                ps_o, vt[sc][:, h * 65:h * 65 + 65],
                                pts[7][:, u * 512:(u + 1) * 512],
                                start=False, stop=(sc == 15),
                                skip_group_check=True)
                        rt = wkp.tile([65, 512], F32, tag="rt", name="rt")
                        nc.vector.reciprocal(rt[64:65, :], ps_o[64:65, :])
                        rtr = wkp.tile([65, 512], F32R, tag="rtr", name="rtr")
                        nc.scalar.copy(rtr[64:65, :], rt[64:65, :])
                        ps_b = psB.tile([64, 512], F32, tag="b", name="b")
                        nc.tensor.matmul(ps_b, ones_t[64:65, 0:64],
                                         rtr[64:65, :], start=True, stop=True)
                        bc = wkp.tile([64, 512], F32, tag="bc", name="bc")
                        nc.scalar.copy(bc[:], ps_b[:])
                        nc.vector.tensor_mul(onrm[ht][ho:ho + 64, :],
                                             ps_o[0:64, :], bc[:])
                    for half in range(2):
                        stg = sgp.tile([128, 4 * 512], F32, tag="stg",
                                       name="stg")
                        for oc in range(4):
                            och = half * 4 + oc
                            ps_op = psB.tile([128, 512], F32, tag="op",
                                             name="op")
                            for cch in range(4):
                                nc.tensor.matmul(
                                    ps_op,
                                    wo[cch][:, och * 128:(och + 1) * 128],
                                    onrm[cch][:], start=(cch == 0),
                                    stop=(cch == 3))
                            nc.scalar.copy(stg[:, oc * 512:(oc + 1) * 512],
                                           ps_op[:])
                        nc.gpsimd.dma_start(
                            out=OUTT.rearrange("(j p) m -> p j m", p=128)
                            [:, half * 4:(half + 1) * 4,
                             t * 512:(t + 1) * 512],
                            in_=stg[:].rearrange("p (j m) -> p j m", j=4))

            vtp.release()

    nc.compile()
    return nc


_NC_CACHE = {}


def _get_module():
    if "nc" not in _NC_CACHE:
        _NC_CACHE["nc"] = build_module()
    return _NC_CACHE["nc"]


def kernel(q, k, v, mask, Wq, bq, Wk, bk, Wv, bv, Wo, bo, **_ignored):
    q = np.asarray(q, dtype=np.float32)
    k = np.asarray(k, dtype=np.float32)
    v = np.asarray(v, dtype=np.float32)
    mask = np.asarray(mask)
    Wq, Wk, Wv, Wo = (np.asarray(w, dtype=np.float32) for w in (Wq, Wk, Wv, Wo))
    bq, bk, bv, bo = (np.asarray(b_, dtype=np.float32) for b_ in (bq, bk, bv, bo))

    maskT = (np.ascontiguousarray(mask[0, 0].T) != 0).astype(ml_dtypes.bfloat16)
    ones = round_f32r(np.ones((128, 128), np.float32))

    xT = {}
    for b_ in range(B):
        xT[("q", b_)] = round_f32r(np.ascontiguousarray(q[b_].T))
        xT[("k", b_)] = round_f32r(np.ascontiguousarray(k[b_].T))
        xT[("v", b_)] = round_f32r(np.ascontiguousarray(v[b_].T))
    wslice = {}
    for hg in range(2):
        og = hg * OG
        wslice[("q", hg)] = round_f32r(np.ascontiguousarray(Wq[og:og + OG, :].T))
        wslice[("k", hg)] = round_f32r(np.ascontiguousarray(Wk[og:og + OG, :].T))
        wslice[("v", hg)] = round_f32r(np.ascontiguousarray(Wv[og:og + OG, :].T))
        wslice[("o", hg)] = round_f32r(np.ascontiguousarray(Wo[:, og:og + OG].T))

    in_maps = []
    for c in range(NCORES):
        b_, hg = c // 2, c % 2
        og = hg * OG
        in_maps.append({
            "XQT": xT[("q", b_)], "XKT": xT[("k", b_)], "XVT": xT[("v", b_)],
            "WQT": wslice[("q", hg)], "WKT": wslice[("k", hg)],
            "WVT": wslice[("v", hg)], "WOT": wslice[("o", hg)],
            "MASKT": maskT,
            "BQ": bq[og:og + OG].astype(np.float32),
            "BK": bk[og:og + OG].astype(np.float32),
            "BV": round_f32r(bv[og:og + OG].reshape(1, OG)),
            "ONES": ones,
        })

    nc = _get_module()
    res = run_bass_kernel_spmd(nc, in_maps, list(range(NCORES)))

    out = np.empty((B, S, D_MODEL), np.float32)
    for b_ in range(B):
        acc = res.results[2 * b_]["OUTT"] + res.results[2 * b_ + 1]["OUTT"]
        out[b_] = acc.T + bo
    return out



# revision 5
# speedup vs baseline: 1.1638x; 1.1638x over previous
"""Multi-head attention on 8 Trainium2 NeuronCores.

Sharding: data-parallel over batch (4) x tensor-parallel over head-groups (2).
Core c handles batch c//2, heads [8*(c%2), 8*(c%2)+8). Each core computes its
partial out-projection (over its 512 channels); host sums the pair per batch.

Device-side design (per core):
  - all matmul inputs bf16 except nothing (x, W, Q^T, K^T, V, P, attn, Wo)
  - Q^T/K^T [512,2048] ch-major bf16; V [2048, 8h*65] tok-major bf16
    (65th col per head = ones -> softmax denominator rides attn@V)
  - per unit (head, 128-token chunk): scores^T via 16 K=64 matmuls into
    psum [128, 8*128] x2 groups; exp on ACT -> P^T bf16; mask-mul on DVE
  - attn@V token-major: psum[t, 65] slot accumulating 16 s'-chunk matmuls
    (N=65 instead of N=512: half the PE cost of the d-major form)
  - normalize: DVE reciprocal of denom col + tensor_scalar_mul -> bf16
  - PE transpose (identity matmul) back to ch-major, DVE copy -> onrm
  - out-proj: Wo^T chunks x onrm -> psum; gpsimd copy -> DMA
  - schedule: prologue K-och0/Q-och0/V-proj; K/Q och1-3 and out-proj
    blocks stolen into the attention loop's PE slack.
  PSUM: s0,s1 [128,1024] + av [128,1024] + pj0,pj1 [128,512] = 8 banks.
"""
import sys

sys.path.insert(0, "/opt/trn_rl_repo")

import numpy as np
import ml_dtypes

import concourse.bass as bass
import concourse.mybir as mybir
import concourse.tile as tile
from concourse import bacc
from concourse.bass_utils import run_bass_kernel_spmd

D_MODEL = 1024
NUM_HEADS = 16
DK = 64
B, S = 4, 2048
NCORES = 8
OG = 512            # channels per head-group
HPG = 8             # heads per group
IC = D_MODEL // 128  # 8 contraction chunks
NTC = S // 128       # 16 token chunks
F32 = mybir.dt.float32
F32R = mybir.dt.float32r
BF16 = mybir.dt.bfloat16


def round_f32r(x: np.ndarray) -> np.ndarray:
    """Round fp32 to fp32r (11-bit mantissa, round-to-nearest-even)."""
    b = np.ascontiguousarray(x, dtype=np.float32).view(np.uint32).astype(np.uint64)
    lsb = (b >> 12) & 1
    b = (b + 0x7FF + lsb) & 0xFFFFF000
    return (b & 0xFFFFFFFF).astype(np.uint32).view(np.float32)


def build_module():
    nc = bacc.Bacc("TRN2", target_bir_lowering=False, debug=False,
                   num_devices=NCORES)
    XQT = nc.dram_tensor("XQT", [D_MODEL, S], BF16, kind="ExternalInput").ap()
    XKT = nc.dram_tensor("XKT", [D_MODEL, S], BF16, kind="ExternalInput").ap()
    XVT = nc.dram_tensor("XVT", [D_MODEL, S], BF16, kind="ExternalInput").ap()
    WQT = nc.dram_tensor("WQT", [D_MODEL, OG], BF16, kind="ExternalInput").ap()
    WKT = nc.dram_tensor("WKT", [D_MODEL, OG], BF16, kind="ExternalInput").ap()
    WVT = nc.dram_tensor("WVT", [D_MODEL, OG], BF16, kind="ExternalInput").ap()
    WOT = nc.dram_tensor("WOT", [OG, D_MODEL], BF16, kind="ExternalInput").ap()
    # mask tiled [tc, p, sc, t]: M4[tc, p, sc, t] = mask[tc*128+t, sc*128+p]
    M4 = nc.dram_tensor("M4", [NTC, 128, NTC, 128], BF16,
                        kind="ExternalInput").ap()
    BQ = nc.dram_tensor("BQ", [OG], F32, kind="ExternalInput").ap()
    BK = nc.dram_tensor("BK", [OG], F32, kind="ExternalInput").ap()
    BV = nc.dram_tensor("BV", [1, OG], F32R, kind="ExternalInput").ap()
    ONES = nc.dram_tensor("ONES", [1, 128], F32R, kind="ExternalInput").ap()
    IDENT = nc.dram_tensor("IDENT", [128, 128], BF16, kind="ExternalInput").ap()
    OUTT = nc.dram_tensor("OUTT", [D_MODEL, S], F32, kind="ExternalOutput").ap()

    Exp = mybir.ActivationFunctionType.Exp
    Ident = mybir.ActivationFunctionType.Identity
    engs = (nc.sync, nc.scalar, nc.gpsimd)

    with tile.TileContext(nc) as tc:
        pp = tc.alloc_tile_pool(name="persist", bufs=1)
        qkp = tc.alloc_tile_pool(name="qkpool", bufs=1)
        vtp = tc.alloc_tile_pool(name="vtpool", bufs=1)
        xp = tc.alloc_tile_pool(name="xpool", bufs=1)
        wp = tc.alloc_tile_pool(name="wpool", bufs=1)
        mp = tc.alloc_tile_pool(name="mpool", bufs=3)
        ptp = tc.alloc_tile_pool(name="ptpool", bufs=8)
        atp = tc.alloc_tile_pool(name="atpool", bufs=6)
        rvp = tc.alloc_tile_pool(name="rvpool", bufs=6)
        ps = tc.alloc_tile_pool(name="ps", bufs=1, space="PSUM")

        # ---- persistent small tensors ----
        bq_t = pp.tile([128, 4], F32, name="bq_t")
        bk_t = pp.tile([128, 4], F32, name="bk_t")
        bv_t = pp.tile([1, OG], F32R, name="bv_t")
        ones_t = pp.tile([1, 128], F32R, name="ones_t")
        ident_t = pp.tile([128, 128], BF16, name="ident_t")
        nc.sync.dma_start(out=bq_t[:], in_=BQ.rearrange("(j p) -> p j", p=128))
        nc.sync.dma_start(out=bk_t[:], in_=BK.rearrange("(j p) -> p j", p=128))
        nc.sync.dma_start(out=bv_t[:], in_=BV)
        nc.sync.dma_start(out=ones_t[:], in_=ONES)
        nc.sync.dma_start(out=ident_t[:], in_=IDENT)

        # ---- resident projection outputs ----
        qT = [qkp.tile([128, S], BF16, name=f"qT{j}") for j in range(4)]
        kT = [qkp.tile([128, S], BF16, name=f"kT{j}") for j in range(4)]
        vt = [vtp.tile([128, HPG * 65], BF16, name=f"vt{j}") for j in range(16)]

        # ---- q/k x and w inputs ----
        xq = [xp.tile([128, S], BF16, name=f"xq{i}") for i in range(IC)]
        xk = [xp.tile([128, S], BF16, name=f"xk{i}") for i in range(IC)]
        wq = [wp.tile([128, OG], BF16, name=f"wq{i}") for i in range(IC)]
        wk = [wp.tile([128, OG], BF16, name=f"wk{i}") for i in range(IC)]
        for i in range(IC):
            sl = slice(i * 128, (i + 1) * 128)
            engs[i % 3].dma_start(out=xk[i][:], in_=XKT[sl, :])
            engs[(i + 1) % 3].dma_start(out=wk[i][:], in_=WKT[sl, :])
        for i in range(IC):
            sl = slice(i * 128, (i + 1) * 128)
            engs[i % 3].dma_start(out=xq[i][:], in_=XQT[sl, :])
            engs[(i + 1) % 3].dma_start(out=wq[i][:], in_=WQT[sl, :])

        # ---- mask tiles (keyed by tcc, reloaded each hp pass) ----
        mtiles = {}

        def load_mask(tcc):
            mh = mp.tile([128, S], BF16, tag="mask", name="mask")
            engs[tcc % 3].dma_start(out=mh[:], in_=M4[tcc])
            mtiles[tcc] = mh

        # ---- projection building blocks ----
        pj_ctr = [0]

        def pj_psum():
            t_ = ps.tile([128, 512], F32, tag=f"pj{pj_ctr[0] % 2}",
                         name=f"pj{pj_ctr[0] % 2}")
            pj_ctr[0] += 1
            return t_

        def proj_block(dst, xs, ws, bias, j, r):
            """One (och j, t-round r) block of Q/K projection."""
            pj = pj_psum()
            for i in range(IC):
                nc.tensor.matmul(pj, ws[i][:, j * 128:(j + 1) * 128],
                                 xs[i][:, r * 512:(r + 1) * 512],
                                 start=(i == 0), stop=(i == IC - 1))
            nc.scalar.activation(dst[j][:, r * 512:(r + 1) * 512], pj, Ident,
                                 bias=bias[:, j:j + 1])

        # ---------------- prologue ----------------
        load_mask(0)
        load_mask(1)
        for r in range(4):
            proj_block(kT, xk, wk, bk_t, 0, r)
        for r in range(4):
            proj_block(qT, xq, wq, bq_t, 0, r)

        # V projection (fully in prologue; xv/wv released after)
        xvp = tc.alloc_tile_pool(name="xvpool", bufs=1)
        xv = [xvp.tile([128, S], BF16, name=f"xv{i}") for i in range(IC)]
        wv = [xvp.tile([128, OG], BF16, name=f"wv{i}") for i in range(IC)]
        for i in range(IC):
            sl = slice(i * 128, (i + 1) * 128)
            engs[i % 3].dma_start(out=xv[i][:], in_=XVT[sl, :])
            engs[(i + 1) % 3].dma_start(out=wv[i][:], in_=WVT[sl, :])
        for tch in range(16):
            ocol = vt[tch][:].rearrange("p (h e) -> p h e", h=HPG)[:, :, 64:65]
            nc.vector.memset(ocol, 1.0)
        for tch in range(16):
            pj = pj_psum()
            for i in range(IC):
                nc.tensor.matmul(pj, xv[i][:, tch * 128:(tch + 1) * 128],
                                 wv[i][:], start=(i == 0), stop=False)
            nc.tensor.matmul(pj, ones_t[0:1, :], bv_t[0:1, :],
                             start=False, stop=True)
            nc.vector.tensor_copy(
                vt[tch][:].rearrange("p (h e) -> p h e", h=HPG)[:, :, 0:64],
                pj[:].rearrange("p (h d) -> p h d", h=HPG))
        xvp.release()

        # pools whose space can reuse the released xv region
        onp = tc.alloc_tile_pool(name="onrmpool", bufs=1)
        onrm = [onp.tile([128, S], BF16, name=f"onrm{c}") for c in range(4)]

        # ---------------- attention loop ----------------
        av = ps.tile([128, 1024], F32, tag="av", name="av")

        def unit_scores(h, tcc):
            ht, ho = h // 2, (h % 2) * 64
            mh = mtiles[tcc]
            pts = []
            for g in range(2):
                s_ps = ps.tile([128, 1024], F32, tag=f"s{g}", name=f"s{g}")
                for sc8 in range(8):
                    sc = g * 8 + sc8
                    nc.tensor.matmul(
                        s_ps[:, sc8 * 128:(sc8 + 1) * 128],
                        kT[ht][ho:ho + 64, sc * 128:(sc + 1) * 128],
                        qT[ht][ho:ho + 64, tcc * 128:(tcc + 1) * 128],
                        start=True, stop=True)
                pt = ptp.tile([128, 1024], BF16, tag="pt", name="pt")
                nc.scalar.activation(pt[:], s_ps, Exp, scale=0.125)
                nc.vector.tensor_mul(pt[:], pt[:],
                                     mh[:, g * 1024:(g + 1) * 1024])
                pts.append(pt)
            return pts

        def unit_attnv(u):
            h, tcc, pts, uidx = u["h"], u["tcc"], u["pts"], u["u"]
            c0 = (uidx % 8) * 128
            for g in range(2):
                for sc8 in range(8):
                    sc = g * 8 + sc8
                    nc.tensor.matmul(
                        av[:, c0:c0 + 65],
                        pts[g][:, sc8 * 128:(sc8 + 1) * 128],
                        vt[sc][:, h * 65:h * 65 + 65],
                        start=(sc == 0), stop=(sc == 15),
                        skip_group_check=True)
            rv = rvp.tile([128, 1], F32, tag="rv", name="rv")
            nc.vector.reciprocal(rv[:], av[:, c0 + 64:c0 + 65])
            at = atp.tile([128, 64], BF16, tag="at", name="at")
            nc.vector.tensor_scalar_mul(at[:], av[:, c0:c0 + 64], rv[:])
            # transpose to ch-major via identity matmul into bf16 psum view
            # (shares the pj psum rotation; pbase alternates with h parity)
            tp = pj_psum()
            tpb = tp[:].bitcast(BF16)
            pbase = (h % 2) * 64
            dst = tpb[pbase:pbase + 64, 0:128]
            nc.tensor.transpose(dst, at[:], ident_t[:])
            nc.vector.tensor_copy(
                onrm[h // 2][(h % 2) * 64:(h % 2) * 64 + 64,
                             tcc * 128:(tcc + 1) * 128], dst)

        def outproj_block(w, och):
            op = pj_psum()
            for cch in range(4):
                nc.tensor.matmul(op, wo[cch][:, och * 128:(och + 1) * 128],
                                 onrm[cch][:, w * 512:(w + 1) * 512],
                                 start=(cch == 0), stop=(cch == 3))
            stg = sgp.tile([128, 512], F32, tag="stg", name="stg")
            nc.gpsimd.tensor_copy(stg[:], op[:])
            nc.gpsimd.dma_start(
                out=OUTT.rearrange("(j p) m -> p j m", p=128)
                [:, och, w * 512:(w + 1) * 512], in_=stg[:])

        steal = []
        for j in (1, 2, 3):
            for r in range(4):
                steal.append(("proj", kT, xk, wk, bk_t, j, r))
            for r in range(4):
                steal.append(("proj", qT, xq, wq, bq_t, j, r))

        wo = None
        sgp = None
        pend = []
        PIPE = 2
        u = 0
        for hp in range(4):
            if hp == 3:
                # out-proj inputs: alloc late so they reuse released x space
                wop = tc.alloc_tile_pool(name="wopool", bufs=1)
                sgp = tc.alloc_tile_pool(name="stgpool", bufs=2)
                wo = [wop.tile([128, D_MODEL], BF16, name=f"wo{j}")
                      for j in range(4)]
                for j in range(4):
                    nc.gpsimd.dma_start(out=wo[j][:],
                                        in_=WOT[j * 128:(j + 1) * 128, :])
            for tcc in range(NTC):
                nxt = hp * NTC + tcc + 2
                if nxt < 4 * NTC:
                    load_mask(nxt % NTC)
                for h in (2 * hp, 2 * hp + 1):
                    pts = unit_scores(h, tcc)
                    pend.append({"h": h, "tcc": tcc, "pts": pts, "u": u})
                    while len(pend) > PIPE:
                        unit_attnv(pend.pop(0))
                    nsteal = 2 if len(steal) > 8 else 1
                    for _ in range(nsteal):
                        if not steal:
                            break
                        it = steal.pop(0)
                        if it[0] == "proj":
                            _, dst, xs, ws, bias, j, r = it
                            proj_block(dst, xs, ws, bias, j, r)
                        else:
                            outproj_block(it[1], it[2])
                    if hp == 3 and h == 7 and tcc % 4 == 3:
                        w = tcc // 4
                        # window w's last unit still pends; outproj blocks
                        # go through the steal queue (deps via framework)
                        for och in range(8):
                            steal.append(("outproj", w, och))
                    u += 1
        while pend:
            unit_attnv(pend.pop(0))
        while steal:
            it = steal.pop(0)
            if it[0] == "proj":
                _, dst, xs, ws, bias, j, r = it
                proj_block(dst, xs, ws, bias, j, r)
            else:
                outproj_block(it[1], it[2])

        # release pools in reverse alloc (stack) order
        for pool in (sgp, wop, onp, ps, rvp, atp, ptp, mp, wp, xp,
                     vtp, qkp, pp):
            if pool is not None:
                pool.release()

    nc.compile()
    return nc


_NC_CACHE = {}


def _get_module():
    if "nc" not in _NC_CACHE:
        _NC_CACHE["nc"] = build_module()
    return _NC_CACHE["nc"]


def kernel(q, k, v, mask, Wq, bq, Wk, bk, Wv, bv, Wo, bo, **_ignored):
    q = np.asarray(q, dtype=np.float32)
    k = np.asarray(k, dtype=np.float32)
    v = np.asarray(v, dtype=np.float32)
    mask = np.asarray(mask)
    Wq, Wk, Wv, Wo = (np.asarray(w, dtype=np.float32) for w in (Wq, Wk, Wv, Wo))
    bq, bk, bv, bo = (np.asarray(b_, dtype=np.float32) for b_ in (bq, bk, bv, bo))

    bf = ml_dtypes.bfloat16
    m = (mask[0, 0] != 0).astype(bf)
    m4 = np.ascontiguousarray(
        m.reshape(NTC, 128, NTC, 128).transpose(0, 3, 2, 1))
    ones = round_f32r(np.ones((1, 128), np.float32))
    ident = np.eye(128, dtype=bf)

    xT = {}
    for b_ in range(B):
        xT[("q", b_)] = np.ascontiguousarray(q[b_].T).astype(bf)
        xT[("k", b_)] = np.ascontiguousarray(k[b_].T).astype(bf)
        xT[("v", b_)] = np.ascontiguousarray(v[b_].T).astype(bf)
    wslice = {}
    for hg in range(2):
        og = hg * OG
        wslice[("q", hg)] = np.ascontiguousarray(Wq[og:og + OG, :].T).astype(bf)
        wslice[("k", hg)] = np.ascontiguousarray(Wk[og:og + OG, :].T).astype(bf)
        wslice[("v", hg)] = np.ascontiguousarray(Wv[og:og + OG, :].T).astype(bf)
        wslice[("o", hg)] = np.ascontiguousarray(Wo[:, og:og + OG].T).astype(bf)

    in_maps = []
    for c in range(NCORES):
        b_, hg = c // 2, c % 2
        og = hg * OG
        in_maps.append({
            "XQT": xT[("q", b_)], "XKT": xT[("k", b_)], "XVT": xT[("v", b_)],
            "WQT": wslice[("q", hg)], "WKT": wslice[("k", hg)],
            "WVT": wslice[("v", hg)], "WOT": wslice[("o", hg)],
            "M4": m4,
            "BQ": bq[og:og + OG].astype(np.float32),
            "BK": bk[og:og + OG].astype(np.float32),
            "BV": round_f32r(bv[og:og + OG].reshape(1, OG)),
            "ONES": ones, "IDENT": ident,
        })

    nc = _get_module()
    res = run_bass_kernel_spmd(nc, in_maps, list(range(NCORES)))

    out = np.empty((B, S, D_MODEL), np.float32)
    for b_ in range(B):
        acc = res.results[2 * b_]["OUTT"] + res.results[2 * b_ + 1]["OUTT"]
        out[b_] = acc.T + bo
    return out
